# revision 3
# baseline (speedup 1.0000x reference)
"""Trainium2 Bass kernel for nn_AugmentShallow (gnn_message_passing).

Reference computation (per batch b):
    g  = x[b, knn_idx[b]]                       # [N, K, 3] gather
    h  = g @ W1.T + b1                          # [N, K, 128]
    h  = relu(h @ Wc0.T + bc0)                  # [N, K, 128]
    h  = relu(h @ Wc1.T + bc1)                  # [N, K, 128]
    m  = mean_k h                               # [N, 128]
    out = m @ W2.T + b2                         # [N, 256]

Strength reduction used here: every per-(n,k) value depends only on the
gathered point index j = knn_idx[b,n,k], so the MLP is evaluated once per
unique point (N instead of N*K rows):
    p[j] = relu(Weff @ x[j] + beff)   with Weff = Wc0 @ W1 (host-fused)
    q[j] = relu(Wc1 @ p[j] + bc1)
    m[n] = sum_k q[knn[n,k]] with 1/K folded into W2
    out  = m @ (W2/K).T + b2
The gather of 128-dim fp16 q-rows runs on the SWDGE dma_gather path
(SBUF-source transposed gather); the K-sum is PE identity-matmul PSUM
accumulation; trans2 is a plain PE matmul with a rank-1 bias matmul.

Sharding: data-parallel over B — core i owns batch i (8 batches, 8 cores,
knn_idx[b] only references batch b, so no cross-core traffic).
"""

import sys

if "/opt/trn_rl_repo" not in sys.path:
    sys.path.insert(0, "/opt/trn_rl_repo")

import numpy as np

B, N, K = 8, 8192, 12
C_IN, C_HID, C_OUT = 3, 128, 256

CHUNK = 1024                    # output tokens per gather chunk
N_CHUNKS = N // CHUNK           # 8
IDX_PER_CHUNK = CHUNK * K       # 12288
IDX_SLOTS = IDX_PER_CHUNK // 16  # 768 free-dim slots (16-partition wrap)
P_TOK = 512                     # tokens per p-stage matmul
MSUB = 512                      # tokens per K-sum psum tile

_CACHE = {}


def _build_program():
    import concourse.bacc as bacc
    import concourse.mybir as mybir
    import concourse.tile as tile

    dt = mybir.dt
    nc = bacc.Bacc("TRN2", target_bir_lowering=False, debug=False, num_devices=8)

    # DRAM I/O (per core = one batch)
    xT_d = nc.dram_tensor("xT", [C_IN, N], dt.float32, kind="ExternalInput")
    idx_d = nc.dram_tensor("idx", [128, N_CHUNKS * IDX_SLOTS], dt.int16,
                           kind="ExternalInput")
    weffT_d = nc.dram_tensor("weffT", [C_IN, C_HID], dt.float32,
                             kind="ExternalInput")
    beff_d = nc.dram_tensor("beff", [C_HID, 1], dt.float32, kind="ExternalInput")
    wc1T_d = nc.dram_tensor("wc1T", [C_HID, C_HID], dt.float16,
                            kind="ExternalInput")
    bc1_d = nc.dram_tensor("bc1r", [1, C_HID], dt.float16, kind="ExternalInput")
    w2T_d = nc.dram_tensor("w2T", [C_HID, C_OUT], dt.float16,
                           kind="ExternalInput")
    b2_d = nc.dram_tensor("b2r", [1, C_OUT], dt.float16, kind="ExternalInput")
    ident_d = nc.dram_tensor("ident", [128, 128], dt.float16,
                             kind="ExternalInput")
    ones_d = nc.dram_tensor("ones", [1, 128], dt.float16, kind="ExternalInput")
    out_d = nc.dram_tensor("out", [N, C_OUT], dt.float32, kind="ExternalOutput")

    with tile.TileContext(nc) as tc:
        with (
            tc.tile_pool(name="const", bufs=1) as cpool,
            tc.tile_pool(name="gpool", bufs=2) as gpool,
            tc.tile_pool(name="mpool", bufs=3) as mpool,
            tc.tile_pool(name="opool", bufs=2) as opool,
            tc.tile_pool(name="pp", bufs=2, space="PSUM") as pp,
            tc.tile_pool(name="pq", bufs=2, space="PSUM") as pq,
            tc.tile_pool(name="pm", bufs=2, space="PSUM") as pm,
            tc.tile_pool(name="po", bufs=2, space="PSUM") as po,
        ):
            # ---- persistent SBUF tensors -------------------------------
            xT = cpool.tile([C_IN, N], dt.float32)
            idx = cpool.tile([128, N_CHUNKS * IDX_SLOTS], dt.int16)
            weffT = cpool.tile([C_IN, C_HID], dt.float32)
            beff = cpool.tile([C_HID, 1], dt.float32)
            wc1T = cpool.tile([C_HID, C_HID], dt.float16)
            bc1 = cpool.tile([1, C_HID], dt.float16)
            w2T = cpool.tile([C_HID, C_OUT], dt.float16)
            b2 = cpool.tile([1, C_OUT], dt.float16)
            ident = cpool.tile([128, 128], dt.float16)
            ones = cpool.tile([1, 128], dt.float16)
            p_f16 = cpool.tile([128, N], dt.float16)   # [ch, tok]
            q_f16 = cpool.tile([128, N], dt.float16)   # token j @ part j%128

            nc.sync.dma_start(xT[:], xT_d.ap()[:])
            nc.sync.dma_start(idx[:], idx_d.ap()[:])
            nc.sync.dma_start(weffT[:], weffT_d.ap()[:])
            nc.sync.dma_start(beff[:], beff_d.ap()[:])
            nc.sync.dma_start(wc1T[:], wc1T_d.ap()[:])
            nc.sync.dma_start(bc1[:], bc1_d.ap()[:])
            nc.sync.dma_start(w2T[:], w2T_d.ap()[:])
            nc.sync.dma_start(b2[:], b2_d.ap()[:])
            nc.sync.dma_start(ident[:], ident_d.ap()[:])
            nc.sync.dma_start(ones[:], ones_d.ap()[:])

            # ---- p = relu(Weff @ x + beff), channel-major [128, N] -----
            for c in range(N // P_TOK):
                ppt = pp.tile([128, P_TOK], dt.float32)
                nc.tensor.matmul(
                    ppt[:], weffT[:], xT[:, c * P_TOK:(c + 1) * P_TOK],
                    start=True, stop=True,
                )
                nc.scalar.activation(
                    p_f16[:, c * P_TOK:(c + 1) * P_TOK], ppt[:],
                    mybir.ActivationFunctionType.Relu, bias=beff[:],
                )

            # ---- q = relu(Wc1 @ p + bc1), token-major tiles ------------
            # psum[tok, ch] = p_tile.T @ wc1T  (+ ones.T @ bc1)
            for t in range(N // 128):
                qpt = pq.tile([128, 128], dt.float32)
                nc.tensor.matmul(
                    qpt[:], p_f16[:, t * 128:(t + 1) * 128], wc1T[:],
                    start=True, stop=False,
                )
                nc.tensor.matmul(qpt[:], ones[:], bc1[:], start=False, stop=True)
                nc.vector.tensor_relu(q_f16[:, t * 128:(t + 1) * 128], qpt[:])

            # ---- gather + K-sum + trans2, chunked ----------------------
            for c in range(N_CHUNKS):
                g = gpool.tile([128, IDX_PER_CHUNK], dt.float16)
                nc.gpsimd.dma_gather(
                    g[:].rearrange("p (a n) -> p a n", a=1),
                    q_f16[:],
                    idx[:, c * IDX_SLOTS:(c + 1) * IDX_SLOTS],
                    num_idxs=IDX_PER_CHUNK,
                    num_idxs_reg=IDX_PER_CHUNK,
                    elem_size=128,
                    transpose=True,
                    sbuf_tokens_per_rank=128,
                    sbuf_free_dim_per_rank=256,
                    single_packet=False,
                )
                m_f16 = mpool.tile([128, CHUNK], dt.float16)  # [ch, tok]
                for j in range(CHUNK // MSUB):
                    mps = pm.tile([128, MSUB], dt.float32)
                    for k in range(K):
                        nc.tensor.matmul(
                            mps[:], ident[:],
                            g[:, k * CHUNK + j * MSUB:k * CHUNK + (j + 1) * MSUB],
                            start=(k == 0), stop=(k == K - 1),
                        )
                    nc.vector.tensor_copy(
                        m_f16[:, j * MSUB:(j + 1) * MSUB], mps[:])

                osb = opool.tile([128, CHUNK // 128 * C_OUT], dt.float32)
                for s in range(CHUNK // 128):
                    ops = po.tile([128, C_OUT], dt.float32)
                    nc.tensor.matmul(
                        ops[:], m_f16[:, s * 128:(s + 1) * 128], w2T[:],
                        start=True, stop=False,
                    )
                    nc.tensor.matmul(ops[:], ones[:], b2[:],
                                     start=False, stop=True)
                    nc.vector.tensor_copy(
                        osb[:, s * C_OUT:(s + 1) * C_OUT], ops[:])
                nc.sync.dma_start(
                    out_d.ap()[c * CHUNK:(c + 1) * CHUNK, :]
                    .rearrange("(s p) o -> p s o", p=128),
                    osb[:].rearrange("p (s o) -> p s o", o=C_OUT),
                )

    nc.compile()
    return nc


def _get_program():
    if "nc" not in _CACHE:
        _CACHE["nc"] = _build_program()
    return _CACHE["nc"]


def _host_prep(x, knn_idx, W1, b1, Wc0, bc0, Wc1, bc1, W2, b2):
    """Fuse weights and build per-core input maps."""
    f64 = np.float64
    weff = (Wc0.astype(f64) @ W1.astype(f64))          # [128, 3]
    beff = (Wc0.astype(f64) @ b1.astype(f64) + bc0.astype(f64))  # [128]
    w2s = W2.astype(f64) / K                            # fold 1/K mean

    weffT = np.ascontiguousarray(weff.T.astype(np.float32))        # [3, 128]
    beff_c = np.ascontiguousarray(beff.astype(np.float32)[:, None])  # [128,1]
    wc1T = np.ascontiguousarray(Wc1.T.astype(np.float16))          # [128,128]
    bc1_r = np.ascontiguousarray(bc1.astype(np.float16)[None, :])  # [1,128]
    w2T = np.ascontiguousarray(w2s.T.astype(np.float16))           # [128,256]
    b2_r = np.ascontiguousarray(b2.astype(np.float16)[None, :])    # [1,256]
    ident = np.eye(128, dtype=np.float16)
    ones = np.ones((1, 128), dtype=np.float16)

    in_maps = []
    for bi in range(B):
        xT = np.ascontiguousarray(x[bi].T.astype(np.float32))      # [3, N]
        # idx layout: per chunk, k-major flat list wrapped into 16
        # partitions, replicated to 128 partitions (8 Q7 core groups).
        kb = knn_idx[bi].astype(np.int16)                          # [N, K]
        cols = []
        for c in range(N_CHUNKS):
            flat = np.ascontiguousarray(
                kb[c * CHUNK:(c + 1) * CHUNK, :].T).reshape(-1)    # k-major
            wrapped = flat.reshape(IDX_SLOTS, 16).T                # [16, S]
            cols.append(np.tile(wrapped, (8, 1)))                  # [128, S]
        idx = np.ascontiguousarray(np.concatenate(cols, axis=1))
        in_maps.append({
            "xT": xT, "idx": idx, "weffT": weffT, "beff": beff_c,
            "wc1T": wc1T, "bc1r": bc1_r, "w2T": w2T, "b2r": b2_r,
            "ident": ident, "ones": ones,
        })
    return in_maps


def kernel(x, knn_idx, W1, b1, Wc0, bc0, Wc1, bc1, W2, b2):
    x = np.asarray(x)
    knn_idx = np.asarray(knn_idx)
    args = [np.asarray(a) for a in (W1, b1, Wc0, bc0, Wc1, bc1, W2, b2)]
    in_maps = _host_prep(x, knn_idx, *args)
    nc = _get_program()
    from concourse import bass_utils
    res = bass_utils.run_bass_kernel_spmd(nc, in_maps, core_ids=list(range(B)))
    return np.stack([res.results[i]["out"] for i in range(B)], axis=0)


# revision 7
# speedup vs baseline: 3.0277x; 3.0277x over previous
"""Trainium2 Bass kernel for nn_AugmentShallow (gnn_message_passing).

Reference computation (per batch b):
    g  = x[b, knn_idx[b]]                       # [N, K, 3] gather
    h  = g @ W1.T + b1                          # [N, K, 128]
    h  = relu(h @ Wc0.T + bc0)                  # [N, K, 128]
    h  = relu(h @ Wc1.T + bc1)                  # [N, K, 128]
    m  = mean_k h                               # [N, 128]
    out = m @ W2.T + b2                         # [N, 256]

Strength reduction used here: every per-(n,k) value depends only on the
gathered point index j = knn_idx[b,n,k], so the MLP is evaluated once per
unique point (N instead of N*K rows):
    p[j] = relu(Weff @ x[j] + beff)   with Weff = Wc0 @ W1 (host-fused)
    q[j] = relu(Wc1 @ p[j] + bc1)
    m[n] = sum_k q[knn[n,k]] with 1/K folded into W2
    out  = m @ (W2/K).T + b2
The gather of 128-dim fp16 q-rows runs on the SWDGE dma_gather path
(SBUF-source transposed gather); the K-sum is PE identity-matmul PSUM
accumulation; trans2 is a plain PE matmul with a rank-1 bias matmul.

Sharding: data-parallel over B — core i owns batch i (8 batches, 8 cores,
knn_idx[b] only references batch b, so no cross-core traffic).
"""

import sys

if "/opt/trn_rl_repo" not in sys.path:
    sys.path.insert(0, "/opt/trn_rl_repo")

import numpy as np

B, N, K = 8, 8192, 12
C_IN, C_HID, C_OUT = 3, 128, 256

CHUNK = 512                     # output tokens per gather chunk
N_CHUNKS = N // CHUNK           # 16
IDX_PER_CHUNK = CHUNK * K       # 6144
IDX_SLOTS = IDX_PER_CHUNK // 16  # 384 free-dim slots (16-partition wrap)
P_TOK = 512                     # tokens per p-stage matmul
MSUB = 512                      # tokens per K-sum psum tile
N_QUEUES = 4                    # SWDGE queues = concurrent Q7 desc-gen pairs

_CACHE = {}


def _build_program():
    import concourse.bacc as bacc
    import concourse.mybir as mybir
    import concourse.tile as tile

    dt = mybir.dt
    nc = bacc.Bacc("TRN2", target_bir_lowering=False, debug=False, num_devices=8,
                   num_swdge_queues=N_QUEUES)

    # DRAM I/O (per core = one batch)
    xT_d = nc.dram_tensor("xT", [C_IN, N], dt.float32, kind="ExternalInput")
    idx_d = nc.dram_tensor("idx", [128, N_CHUNKS * IDX_SLOTS], dt.int16,
                           kind="ExternalInput")
    weffT_d = nc.dram_tensor("weffT", [C_IN, C_HID], dt.float32,
                             kind="ExternalInput")
    beff_d = nc.dram_tensor("beff", [C_HID, 1], dt.float32, kind="ExternalInput")
    wc1T_d = nc.dram_tensor("wc1T", [C_HID, C_HID], dt.float16,
                            kind="ExternalInput")
    bc1_d = nc.dram_tensor("bc1r", [1, C_HID], dt.float16, kind="ExternalInput")
    w2T_d = nc.dram_tensor("w2T", [C_HID, C_OUT], dt.float16,
                           kind="ExternalInput")
    b2_d = nc.dram_tensor("b2r", [1, C_OUT], dt.float16, kind="ExternalInput")
    ident_d = nc.dram_tensor("ident", [128, 128], dt.float16,
                             kind="ExternalInput")
    ones_d = nc.dram_tensor("ones", [1, 128], dt.float16, kind="ExternalInput")
    out_d = nc.dram_tensor("out", [N, C_OUT], dt.float32, kind="ExternalOutput")

    with tile.TileContext(nc) as tc:
        with (
            tc.tile_pool(name="const", bufs=1) as cpool,
            tc.tile_pool(name="gpool", bufs=6) as gpool,
            tc.tile_pool(name="mpool", bufs=3) as mpool,
            tc.tile_pool(name="opool", bufs=2) as opool,
            tc.tile_pool(name="pp", bufs=2, space="PSUM") as pp,
            tc.tile_pool(name="pq", bufs=2, space="PSUM") as pq,
            tc.tile_pool(name="pm", bufs=2, space="PSUM") as pm,
            tc.tile_pool(name="po", bufs=2, space="PSUM") as po,
        ):
            # ---- persistent SBUF tensors -------------------------------
            xT = cpool.tile([C_IN, N], dt.float32)
            idx = cpool.tile([128, N_CHUNKS * IDX_SLOTS], dt.int16)
            weffT = cpool.tile([C_IN, C_HID], dt.float32)
            beff = cpool.tile([C_HID, 1], dt.float32)
            wc1T = cpool.tile([C_HID, C_HID], dt.float16)
            bc1 = cpool.tile([1, C_HID], dt.float16)
            w2T = cpool.tile([C_HID, C_OUT], dt.float16)
            b2 = cpool.tile([1, C_OUT], dt.float16)
            ident = cpool.tile([128, 128], dt.float16)
            ones = cpool.tile([1, 128], dt.float16)
            p_f16 = cpool.tile([128, N], dt.float16)   # [ch, tok]
            q_f16 = cpool.tile([128, N], dt.float16)   # token j @ part j%128

            nc.sync.dma_start(xT[:], xT_d.ap()[:])
            nc.sync.dma_start(idx[:], idx_d.ap()[:])
            nc.sync.dma_start(weffT[:], weffT_d.ap()[:])
            nc.sync.dma_start(beff[:], beff_d.ap()[:])
            nc.sync.dma_start(wc1T[:], wc1T_d.ap()[:])
            nc.sync.dma_start(bc1[:], bc1_d.ap()[:])
            nc.sync.dma_start(w2T[:], w2T_d.ap()[:])
            nc.sync.dma_start(b2[:], b2_d.ap()[:])
            nc.sync.dma_start(ident[:], ident_d.ap()[:])
            nc.sync.dma_start(ones[:], ones_d.ap()[:])

            # ---- p = relu(Weff @ x + beff), channel-major [128, N] -----
            for c in range(N // P_TOK):
                ppt = pp.tile([128, P_TOK], dt.float32)
                nc.tensor.matmul(
                    ppt[:], weffT[:], xT[:, c * P_TOK:(c + 1) * P_TOK],
                    start=True, stop=True,
                )
                nc.scalar.activation(
                    p_f16[:, c * P_TOK:(c + 1) * P_TOK], ppt[:],
                    mybir.ActivationFunctionType.Relu, bias=beff[:],
                )

            # ---- q = relu(Wc1 @ p + bc1), token-major tiles ------------
            # psum[tok, ch] = p_tile.T @ wc1T  (+ ones.T @ bc1)
            for t in range(N // 128):
                qpt = pq.tile([128, 128], dt.float32)
                nc.tensor.matmul(
                    qpt[:], p_f16[:, t * 128:(t + 1) * 128], wc1T[:],
                    start=True, stop=False,
                )
                nc.tensor.matmul(qpt[:], ones[:], bc1[:], start=False, stop=True)
                nc.vector.tensor_relu(q_f16[:, t * 128:(t + 1) * 128], qpt[:])

            # ---- gather + K-sum + trans2, chunked ----------------------
            for c in range(N_CHUNKS):
                g = gpool.tile([128, IDX_PER_CHUNK], dt.float16)
                nc.gpsimd.dma_gather(
                    g[:].rearrange("p (a n) -> p a n", a=1),
                    q_f16[:],
                    idx[:, c * IDX_SLOTS:(c + 1) * IDX_SLOTS],
                    num_idxs=IDX_PER_CHUNK,
                    num_idxs_reg=IDX_PER_CHUNK,
                    elem_size=128,
                    transpose=True,
                    sbuf_tokens_per_rank=128,
                    sbuf_free_dim_per_rank=256,
                    single_packet=False,
                    queue_num=c % N_QUEUES,
                )
                m_f16 = mpool.tile([128, CHUNK], dt.float16)  # [ch, tok]
                for j in range(CHUNK // MSUB):
                    mps = pm.tile([128, MSUB], dt.float32)
                    for k in range(K):
                        nc.tensor.matmul(
                            mps[:], ident[:],
                            g[:, k * CHUNK + j * MSUB:k * CHUNK + (j + 1) * MSUB],
                            start=(k == 0), stop=(k == K - 1),
                        )
                    nc.vector.tensor_copy(
                        m_f16[:, j * MSUB:(j + 1) * MSUB], mps[:])

                osb = opool.tile([128, CHUNK // 128 * C_OUT], dt.float32)
                for s in range(CHUNK // 128):
                    ops = po.tile([128, C_OUT], dt.float32)
                    nc.tensor.matmul(
                        ops[:], m_f16[:, s * 128:(s + 1) * 128], w2T[:],
                        start=True, stop=False,
                    )
                    nc.tensor.matmul(ops[:], ones[:], b2[:],
                                     start=False, stop=True)
                    nc.vector.tensor_copy(
                        osb[:, s * C_OUT:(s + 1) * C_OUT], ops[:])
                nc.sync.dma_start(
                    out_d.ap()[c * CHUNK:(c + 1) * CHUNK, :]
                    .rearrange("(s p) o -> p s o", p=128),
                    osb[:].rearrange("p (s o) -> p s o", o=C_OUT),
                )

    nc.compile()
    return nc


def _get_program():
    if "nc" not in _CACHE:
        _CACHE["nc"] = _build_program()
    return _CACHE["nc"]


def _host_prep(x, knn_idx, W1, b1, Wc0, bc0, Wc1, bc1, W2, b2):
    """Fuse weights and build per-core input maps."""
    f64 = np.float64
    weff = (Wc0.astype(f64) @ W1.astype(f64))          # [128, 3]
    beff = (Wc0.astype(f64) @ b1.astype(f64) + bc0.astype(f64))  # [128]
    w2s = W2.astype(f64) / K                            # fold 1/K mean

    weffT = np.ascontiguousarray(weff.T.astype(np.float32))        # [3, 128]
    beff_c = np.ascontiguousarray(beff.astype(np.float32)[:, None])  # [128,1]
    wc1T = np.ascontiguousarray(Wc1.T.astype(np.float16))          # [128,128]
    bc1_r = np.ascontiguousarray(bc1.astype(np.float16)[None, :])  # [1,128]
    w2T = np.ascontiguousarray(w2s.T.astype(np.float16))           # [128,256]
    b2_r = np.ascontiguousarray(b2.astype(np.float16)[None, :])    # [1,256]
    ident = np.eye(128, dtype=np.float16)
    ones = np.ones((1, 128), dtype=np.float16)

    in_maps = []
    for bi in range(B):
        xT = np.ascontiguousarray(x[bi].T.astype(np.float32))      # [3, N]
        # idx layout: per chunk, k-major flat list wrapped into 16
        # partitions, replicated to 128 partitions (8 Q7 core groups).
        kb = knn_idx[bi].astype(np.int16)                          # [N, K]
        cols = []
        for c in range(N_CHUNKS):
            flat = np.ascontiguousarray(
                kb[c * CHUNK:(c + 1) * CHUNK, :].T).reshape(-1)    # k-major
            wrapped = flat.reshape(IDX_SLOTS, 16).T                # [16, S]
            cols.append(np.tile(wrapped, (8, 1)))                  # [128, S]
        idx = np.ascontiguousarray(np.concatenate(cols, axis=1))
        in_maps.append({
            "xT": xT, "idx": idx, "weffT": weffT, "beff": beff_c,
            "wc1T": wc1T, "bc1r": bc1_r, "w2T": w2T, "b2r": b2_r,
            "ident": ident, "ones": ones,
        })
    return in_maps


def kernel(x, knn_idx, W1, b1, Wc0, bc0, Wc1, bc1, W2, b2):
    x = np.asarray(x)
    knn_idx = np.asarray(knn_idx)
    args = [np.asarray(a) for a in (W1, b1, Wc0, bc0, Wc1, bc1, W2, b2)]
    in_maps = _host_prep(x, knn_idx, *args)
    nc = _get_program()
    from concourse import bass_utils
    res = bass_utils.run_bass_kernel_spmd(nc, in_maps, core_ids=list(range(B)))
    return np.stack([res.results[i]["out"] for i in range(B)], axis=0)
